# revision 4
# baseline (speedup 1.0000x reference)
"""Causal attention (B=4, S=2048, D=1024, single head) on 8 TRN2 NeuronCores.

Sharding: data-parallel over batch (4 pairs of cores); within each pair
the K/V context is split by interleaved 128-row chunks (core parity p
owns global chunks {2j+p}).  Each core projects Q/K/V for its own 1024
rows.  Attention q-columns are processed in a RANK-RELATIVE gathered
order [own 8 blocks | peer 8 blocks]: the own half's Q is read straight
from the SBUF staging tile (no communication), and the peer half is
recovered from a pairwise AllReduce(add) of the Q staging buffers via
one local subtract (peer = sum - own, exact to ~1 ulp bf16).  Each core
computes causal score blocks of all 2048 gathered q-columns against its
own 1024-row context and emits *unnormalized* partial attention output
plus per-column partial softmax denominators (computed off the PE on
GpSimd via partition_all_reduce); the host adds the pair's partials and
normalizes.

PE-stream optimizations: dummy warm-up matmuls cover the tensor-engine
p-state ramp; a single PSUM pool with per-bank tags avoids pool-
transition drains; inputs are host-packed partition-major so every DMA
moves 2KB+ per-partition lines; causal masks cover only the 128-column
diagonal block of each masked score tile; AV blocks run in ascending
causal size so they never wait on the last exp; the final tile's output
DMAs are split per 512-column half to shrink the tail.
"""

import sys

if "/opt/trn_rl_repo" not in sys.path:
    sys.path.insert(0, "/opt/trn_rl_repo")

import ml_dtypes
import numpy as np

import concourse.bacc as bacc
import concourse.bass_isa as bass_isa
import concourse.tile as tile
from concourse import mybir
from concourse.bass_utils import run_bass_kernel_spmd

# bass_utils imports antenv.axon_hooks when tracing is requested (e.g. via a
# BASS_TRACE env var); the image's antenv lacks that module, so provide a
# no-op fallback rather than crashing.
try:
    import antenv.axon_hooks  # noqa: F401
except ImportError:
    import types as _types

    _ah = _types.ModuleType("antenv.axon_hooks")
    _ah._hook = None
    _ah.set_axon_ntff_profile_hook = lambda h: setattr(_ah, "_hook", h)
    _ah.get_axon_ntff_profile_hook = lambda: _ah._hook
    sys.modules["antenv.axon_hooks"] = _ah

B, S, D = 4, 2048, 1024
NB = S // 128          # 16 q-blocks of 128 per batch
NT = S // 512          # 4 q-tiles of 512
IC = D // 128          # 8 contraction chunks
OC = D // 128          # 8 output-dim chunks
LC = 8                 # local k-chunks per core (S/2/128)
SCALE = 1.0 / np.sqrt(D)  # 0.03125
# gathered tiles: 0 = own st0, 1 = own st1, 2 = peer st0, 3 = peer st1
NJ_TILE = [4, 8, 4, 8]  # local k-chunks needed per gathered q-tile

BF16 = mybir.dt.bfloat16
F32 = mybir.dt.float32

_module_cache = None
last_results = None  # BassKernelResults of the most recent run (for test harness)


def _masked_js(tt):
    """Local chunk indices whose score blocks carry an offset+mask."""
    return range(4) if tt in (0, 2) else range(4, 8)


def _build_module():
    nc = bacc.Bacc("TRN2", target_bir_lowering=False, debug=False, num_devices=8)
    # all inputs packed partition-major on the host: per-partition lines are
    # 2KB+ contiguous so every input DMA runs at full HBM efficiency
    xP = nc.dram_tensor("xP", [128, IC * 1024], BF16, kind="ExternalInput").ap()
    wqP = nc.dram_tensor("wqP", [128, IC * 1024], BF16, kind="ExternalInput").ap()
    wkP = nc.dram_tensor("wkP", [128, IC * 1024], BF16, kind="ExternalInput").ap()
    wvP = nc.dram_tensor("wvP", [128, IC * 1024], BF16, kind="ExternalInput").ap()
    mskP = nc.dram_tensor("mskP", [128, 2 * 128], BF16, kind="ExternalInput").ap()
    out_p = nc.dram_tensor("out_p", [S, D], F32, kind="ExternalOutput").ap()
    rs_out = nc.dram_tensor("rs_out", [1, S], F32, kind="ExternalOutput").ap()

    with tile.TileContext(nc) as tc:
        with (
            tc.tile_pool(name="wp", bufs=1) as wp,
            tc.tile_pool(name="xp", bufs=1) as xp,
            tc.tile_pool(name="kqv", bufs=1) as kqv,
            tc.tile_pool(name="mp", bufs=1) as mp,
            tc.tile_pool(name="ptp", bufs=2) as ptp,
            tc.tile_pool(name="stg", bufs=4) as stg,
            tc.tile_pool(name="dacc", bufs=2) as dap,
            tc.tile_pool(name="dr", bufs=1, space="DRAM") as dr,
            tc.tile_pool(name="ps", bufs=1, space="PSUM") as ps,
        ):
            def pbank(o, name):
                return ps.tile([128, 512], F32, tag=f"pb{o}", bufs=1, name=name)

            # ---- PE warm-up: the tensor engine ramps 0.65->2.4GHz over ~3us
            #      of continuous execution; burn the ramp on dummy matmuls
            #      while the first input DMAs are still in flight ----
            wu_src = mp.tile([128, 512], BF16, tag="wusrc", name="wu_src")
            nc.vector.memset(wu_src, 0.0)
            wu_ps = pbank(7, "wu_ps")
            for w in range(6):
                nc.tensor.matmul(
                    wu_ps, lhsT=wu_src[:, 0:128], rhs=wu_src,
                    start=(w == 0), stop=(w == 5),
                )
            nc.vector.tensor_copy(wu_src, wu_ps)

            xt_sb = [
                xp.tile([128, 1024], BF16, tag=f"x{i}", name=f"x{i}")
                for i in range(IC)
            ]
            wq_sb = [
                wp.tile([128, D], BF16, tag=f"wq{i}", name=f"wq{i}") for i in range(IC)
            ]
            wk_sb = [
                wp.tile([128, D], BF16, tag=f"wk{i}", name=f"wk{i}") for i in range(IC)
            ]
            wv_sb = [
                wp.tile([128, D], BF16, tag=f"wv{i}", name=f"wv{i}") for i in range(IC)
            ]
            # wave 1: x on sync queue, Wq on scalar queue, in consumption
            # order; the very first chunks are split so matmul #1 can start
            # as soon as ~300KB has landed
            nc.sync.dma_start(xt_sb[0][:, 0:512], xP[:, 0:512])
            nc.scalar.dma_start(wq_sb[0][:, 0:128], wqP[:, 0:128])
            nc.sync.dma_start(xt_sb[0][:, 512:1024], xP[:, 512:1024])
            nc.scalar.dma_start(wq_sb[0][:, 128:1024], wqP[:, 128:1024])
            for i in range(1, IC):
                nc.sync.dma_start(xt_sb[i], xP[:, 1024 * i : 1024 * (i + 1)])
                nc.scalar.dma_start(wq_sb[i], wqP[:, 1024 * i : 1024 * (i + 1)])
            # wave 2: Wk on scalar, Wv on sync
            for i in range(IC):
                nc.scalar.dma_start(wk_sb[i], wkP[:, 1024 * i : 1024 * (i + 1)])
                nc.sync.dma_start(wv_sb[i], wvP[:, 1024 * i : 1024 * (i + 1)])
            mask_all = mp.tile([128, 2, 128], BF16, tag="masks", name="masks")
            nc.scalar.dma_start(mask_all, mskP.rearrange("p (m c) -> p m c", c=128))

            kt_sb = [kqv.tile([128, S // 2], BF16, tag=f"kt{o}", name=f"kt{o}") for o in range(OC)]
            vn_sb = [kqv.tile([128, D], BF16, tag=f"vn{j}", name=f"vn{j}") for j in range(LC)]
            # own Q staging: doubles as the scores rhs for tiles 0/1
            qs = kqv.tile([128, 2, OC, 512], BF16, tag="qs", name="qs")
            # peer Q, recovered as (own+peer) - own after the AllReduce
            qt_peer = kqv.tile([128, 2, OC, 512], BF16, tag="qtp", name="qt_peer")
            rs_sb = mp.tile([1, S], F32, tag="rs", name="rs")

            # DRAM bounce buffers for the pairwise Q AllReduce
            qhalf = dr.tile([128, 2 * 4096], BF16, name="qhalf")
            qsum = dr.tile([128, 2 * 4096], BF16, name="qsum")

            def copy_to(dst):
                # projection copies go to DVE: the Scalar engine is busy
                # issuing input DMAs early on
                return lambda pp: nc.vector.tensor_copy(dst, pp)

            def proj_iouter(lhs_slices, rhs_slices, dsts, pname):
                pps = [pbank(o, f"{pname}{o}") for o in range(len(dsts))]
                for i in range(IC):
                    for o in range(len(dsts)):
                        nc.tensor.matmul(
                            pps[o],
                            lhsT=lhs_slices(i, o),
                            rhs=rhs_slices(i, o),
                            start=(i == 0),
                            stop=(i == IC - 1),
                        )
                for o, dst in enumerate(dsts):
                    dst(pps[o])

            # ---- phase 1: Q projection (both 512-column halves of own x),
            #      staged to qs; each half's flat copy streams to DRAM for
            #      the pairwise AllReduce as soon as it completes ----
            for st in range(2):
                proj_iouter(
                    lambda i, o: wq_sb[i][:, 128 * o : 128 * (o + 1)],
                    lambda i, o, s=st: xt_sb[i][:, 512 * s : 512 * (s + 1)],
                    [copy_to(qs[:, st, o, :]) for o in range(OC)],
                    f"pq{st}",
                )
                nc.gpsimd.dma_start(
                    qhalf[:, 4096 * st : 4096 * (st + 1)],
                    qs[:, st].rearrange("p o c -> p (o c)"),
                )
            # pairwise exchange: AllReduce(add) then peer = sum - own.
            # Triggered from GpSimd (idle until the attention phase).
            nc.gpsimd.collective_compute(
                kind="AllReduce",
                op=mybir.AluOpType.add,
                replica_groups=[[0, 1], [2, 3], [4, 5], [6, 7]],
                ins=[qhalf],
                outs=[qsum],
            )
            for st in range(2):
                nc.sync.dma_start(
                    qt_peer[:, st],
                    qsum[:, 4096 * st : 4096 * (st + 1)].rearrange(
                        "p (o c) -> p o c", c=512
                    ),
                )
                nc.vector.tensor_sub(qt_peer[:, st], qt_peer[:, st], qs[:, st])

            # K projection (both halves of the local context)
            for st in range(2):
                proj_iouter(
                    lambda i, o: wk_sb[i][:, 128 * o : 128 * (o + 1)],
                    lambda i, o, s=st: xt_sb[i][:, 512 * s : 512 * (s + 1)],
                    [copy_to(kt_sb[o][:, 512 * st : 512 * (st + 1)]) for o in range(OC)],
                    f"pk{st}",
                )
            # V projection for chunks j<4 (all that attention tiles 0 and 2
            # need); chunks j>=4 are projected between tiles 0 and 1
            proj_iouter(
                lambda i, c: xt_sb[i][:, 128 * (c // 2) : 128 * (c // 2 + 1)],
                lambda i, c: wv_sb[i][:, 512 * (c % 2) : 512 * (c % 2 + 1)],
                [
                    (lambda dst: lambda pp: nc.any.tensor_copy(dst, pp))(
                        vn_sb[c // 2][:, 512 * (c % 2) : 512 * (c % 2 + 1)]
                    )
                    for c in range(8)
                ],
                "pva",
            )

            def vn_late():
                for c in range(8):
                    j, ot = 4 + c // 2, c % 2
                    pp = pbank(c, "pvb")
                    for i in range(IC):
                        nc.tensor.matmul(
                            pp,
                            lhsT=xt_sb[i][:, 128 * j : 128 * (j + 1)],
                            rhs=wv_sb[i][:, 512 * ot : 512 * (ot + 1)],
                            start=(i == 0),
                            stop=(i == IC - 1),
                        )
                    if c % 2 == 0:
                        nc.vector.tensor_copy(
                            vn_sb[j][:, 512 * ot : 512 * (ot + 1)], pp
                        )
                    else:
                        nc.scalar.copy(
                            vn_sb[j][:, 512 * ot : 512 * (ot + 1)], pp
                        )

            # ---- phase 2: attention over gathered q-tiles ----
            def attention_tile(tt):
                st = tt % 2
                own = tt < 2
                q_src = qs[:, st] if own else qt_peer[:, st]
                nj = NJ_TILE[tt]
                masked = set(_masked_js(tt))
                mgrp = 0 if own else 1
                pt_tiles = []
                offs = []
                acc = dap.tile([128, 512], F32, tag="dacc", name=f"acc{tt}")
                for j in range(nj):
                    # in a masked block the first 128*(j%4) q-columns are
                    # fully masked out -- skip computing them entirely; the
                    # mask itself only covers the 128-wide diagonal block
                    off = 128 * (j % 4) if j in masked else 0
                    offs.append(off)
                    sp = pbank(j, f"sc{tt}_{j}")
                    for o in range(OC):
                        nc.tensor.matmul(
                            sp[:, off:512],
                            lhsT=kt_sb[o][:, 128 * j : 128 * (j + 1)],
                            rhs=q_src[:, o, off:512],
                            start=(o == 0),
                            stop=(o == OC - 1),
                        )
                    pt = ptp.tile([128, 512], BF16, tag=f"pt{j}", name=f"pt{tt}_{j}")
                    nc.scalar.activation(
                        pt[:, off:512],
                        sp[:, off:512],
                        mybir.ActivationFunctionType.Exp,
                        scale=SCALE,
                    )
                    if j in masked:
                        nc.vector.tensor_mul(
                            pt[:, off : off + 128],
                            pt[:, off : off + 128],
                            mask_all[:, mgrp, :],
                        )
                    pt_tiles.append(pt)
                    # partial softmax denominators accumulate on GpSimd
                    # (j=0 always covers the full 512 columns)
                    if j == 0:
                        nc.gpsimd.tensor_copy(acc, pt)
                    else:
                        nc.gpsimd.tensor_add(
                            acc[:, off:512], acc[:, off:512], pt[:, off:512]
                        )
                red = dap.tile([128, 512], F32, tag="dred", name=f"red{tt}")
                nc.gpsimd.partition_all_reduce(
                    red, acc, channels=128, reduce_op=bass_isa.ReduceOp.add
                )
                nc.vector.tensor_copy(rs_sb[:, 512 * tt : 512 * (tt + 1)], red[0:1, :])

                # AV in ascending causal size: the first blocks depend only on
                # early pt tiles, so they never wait on the last exp
                bank = 0 if nj == 8 else 4
                for qq in range(4):
                    qbg = 4 * tt + qq
                    njs = 4 * st + qq + 1
                    ost = stg.tile([128, D], F32, tag="ost", name=f"ost{tt}_{qq}")
                    for ot in range(2):
                        apsum = pbank(bank % 8, f"at{tt}_{qq}_{ot}")
                        bank += 1
                        for j in range(njs):
                            nc.tensor.matmul(
                                apsum,
                                lhsT=pt_tiles[j][:, 128 * qq : 128 * (qq + 1)],
                                rhs=vn_sb[j][:, 512 * ot : 512 * (ot + 1)],
                                start=(j == 0),
                                stop=(j == njs - 1),
                            )
                        nc.vector.tensor_copy(ost[:, 512 * ot : 512 * (ot + 1)], apsum)
                        if tt == 3:
                            # final tile: split per-half on the idle Scalar
                            # queue so the last DMA is only 256KB
                            nc.scalar.dma_start(
                                out_p[128 * qbg : 128 * (qbg + 1), 512 * ot : 512 * (ot + 1)],
                                ost[:, 512 * ot : 512 * (ot + 1)],
                            )
                    if tt != 3:
                        nc.gpsimd.dma_start(
                            out_p[128 * qbg : 128 * (qbg + 1), :], ost
                        )

            attention_tile(0)
            vn_late()
            attention_tile(1)
            attention_tile(2)
            attention_tile(3)

            nc.gpsimd.dma_start(rs_out, rs_sb)

    nc.compile()
    return nc


def _get_module():
    global _module_cache
    if _module_cache is None:
        _module_cache = _build_module()
    return _module_cache


def _gather_pos(par: int) -> np.ndarray:
    """pos[q] = row position of global row q in this core's rank-relative
    gathered order ([own interleaved blocks | peer interleaved blocks])."""
    q = np.arange(S)
    g = q // 128
    r = q % 128
    own = (g % 2) == par
    blk = np.where(own, (g - par) // 2, 8 + (g - (1 - par)) // 2)
    return 128 * blk + r


def _pack_pmajor(a2d: np.ndarray) -> np.ndarray:
    """[IC*128, W] -> [128, IC*W] partition-major packing (bf16)."""
    w = a2d.shape[1]
    return np.ascontiguousarray(
        a2d.reshape(IC, 128, w).transpose(1, 0, 2).reshape(128, IC * w)
    )


def kernel(x, Wq, Wk, Wv, _trace=False):
    global last_results
    nc = _get_module()

    bf = ml_dtypes.bfloat16

    wqP = _pack_pmajor(np.ascontiguousarray(Wq.T).astype(bf))
    wkP = _pack_pmajor(np.ascontiguousarray(Wk.T).astype(bf))
    wvP = _pack_pmajor(np.ascontiguousarray(Wv.T).astype(bf))

    # masks: group 0 = triangular diagonal block (own tiles), group 1 =
    # all-ones (parity 0) / all-zeros (parity 1) for the peer tiles' block
    tri = (np.arange(128)[None, :] >= np.arange(128)[:, None]).astype(np.float32)
    msks = []
    for par in range(2):
        m = np.zeros((128, 2, 128), dtype=np.float32)
        m[:, 0, :] = tri
        m[:, 1, :] = 1.0 - par
        msks.append(m.reshape(128, 256).astype(bf))

    # per-parity column selection: core owns global k-chunks {2j+par}
    own_cols = [
        (128 * (2 * np.arange(LC)[:, None] + par) + np.arange(128)[None, :]).reshape(-1)
        for par in range(2)
    ]

    in_maps = []
    for c in range(8):
        b, par = c // 2, c % 2
        xTb = x[b].T[:, own_cols[par]].astype(bf)  # [D, S//2]
        in_maps.append(
            {
                "xP": _pack_pmajor(xTb),
                "wqP": wqP,
                "wkP": wkP,
                "wvP": wvP,
                "mskP": msks[par],
            }
        )

    kwargs = {}
    if _trace:
        kwargs["trace"] = True
    res = run_bass_kernel_spmd(nc, in_maps, core_ids=list(range(8)), **kwargs)
    last_results = res

    pos = [_gather_pos(0), _gather_pos(1)]
    out = np.empty((B, S, D), dtype=np.float32)
    for b in range(B):
        rA = res.results[2 * b]
        rB = res.results[2 * b + 1]
        num = rA["out_p"][pos[0]] + rB["out_p"][pos[1]]
        den = rA["rs_out"][0][pos[0]] + rB["rs_out"][0][pos[1]]
        out[b] = num / den[:, None]
    return out


# revision 7
# speedup vs baseline: 1.2094x; 1.2094x over previous
"""Causal attention (B=4, S=2048, D=1024, single head) on 8 TRN2 NeuronCores.

Sharding: data-parallel over batch (4 pairs of cores); within each pair
the K/V context is split by interleaved 128-row chunks (core parity p
owns global chunks {2j+p}).  Each core projects Q/K/V for its own 1024
rows.  Attention q-columns are processed in a RANK-RELATIVE gathered
order [own 8 blocks | peer 8 blocks]: the own half's Q is read straight
from the SBUF staging tile (no communication), and the peer half is
recovered from a pairwise AllReduce(add) of the Q staging buffers via
one local subtract (peer = sum - own, exact to ~1 ulp bf16).  Each core
computes causal score blocks of all 2048 gathered q-columns against its
own 1024-row context and emits *unnormalized* partial attention output
plus per-column partial softmax denominators (computed off the PE on
GpSimd via partition_all_reduce); the host adds the pair's partials and
normalizes.

PE-stream optimizations: dummy warm-up matmuls cover the tensor-engine
p-state ramp; a single PSUM pool with per-bank tags avoids pool-
transition drains; inputs are host-packed partition-major so every DMA
moves 2KB+ per-partition lines; causal masks cover only the 128-column
diagonal block of each masked score tile; AV blocks run in ascending
causal size so they never wait on the last exp; the final tile's output
DMAs are split per 512-column half to shrink the tail.
"""

import sys

if "/opt/trn_rl_repo" not in sys.path:
    sys.path.insert(0, "/opt/trn_rl_repo")

import ml_dtypes
import numpy as np

import concourse.bacc as bacc
import concourse.bass_isa as bass_isa
import concourse.tile as tile
from concourse import mybir
from concourse.bass_utils import run_bass_kernel_spmd

# bass_utils imports antenv.axon_hooks when tracing is requested (e.g. via a
# BASS_TRACE env var); the image's antenv lacks that module, so provide a
# no-op fallback rather than crashing.
try:
    import antenv.axon_hooks  # noqa: F401
except ImportError:
    import types as _types

    _ah = _types.ModuleType("antenv.axon_hooks")
    _ah._hook = None
    _ah.set_axon_ntff_profile_hook = lambda h: setattr(_ah, "_hook", h)
    _ah.get_axon_ntff_profile_hook = lambda: _ah._hook
    sys.modules["antenv.axon_hooks"] = _ah

B, S, D = 4, 2048, 1024
NB = S // 128          # 16 q-blocks of 128 per batch
NT = S // 512          # 4 q-tiles of 512
IC = D // 128          # 8 contraction chunks
OC = D // 128          # 8 output-dim chunks
LC = 8                 # local k-chunks per core (S/2/128)
SCALE = 1.0 / np.sqrt(D)  # 0.03125
# gathered tiles: 0 = own st0, 1 = own st1, 2 = peer st0, 3 = peer st1
NJ_TILE = [4, 8, 4, 8]  # local k-chunks needed per gathered q-tile

BF16 = mybir.dt.bfloat16
F32 = mybir.dt.float32

_module_cache = None
last_results = None  # BassKernelResults of the most recent run (for test harness)


def _masked_js(tt):
    """Local chunk indices whose score blocks carry an offset+mask."""
    return range(4) if tt in (0, 2) else range(4, 8)


def _build_module():
    nc = bacc.Bacc("TRN2", target_bir_lowering=False, debug=False, num_devices=8)
    # all inputs packed partition-major on the host: per-partition lines are
    # 2KB+ contiguous so every input DMA runs at full HBM efficiency
    xP = nc.dram_tensor("xP", [128, IC * 1024], BF16, kind="ExternalInput").ap()
    wqP = nc.dram_tensor("wqP", [128, IC * 1024], BF16, kind="ExternalInput").ap()
    wkP = nc.dram_tensor("wkP", [128, IC * 1024], BF16, kind="ExternalInput").ap()
    wvP = nc.dram_tensor("wvP", [128, IC * 1024], BF16, kind="ExternalInput").ap()
    mskP = nc.dram_tensor("mskP", [128, 2 * 128], BF16, kind="ExternalInput").ap()
    out_p = nc.dram_tensor("out_p", [S, D], F32, kind="ExternalOutput").ap()
    rs_out = nc.dram_tensor("rs_out", [1, S], F32, kind="ExternalOutput").ap()

    with tile.TileContext(nc) as tc:
        with (
            tc.tile_pool(name="wp", bufs=1) as wp,
            tc.tile_pool(name="xp", bufs=1) as xp,
            tc.tile_pool(name="kqv", bufs=1) as kqv,
            tc.tile_pool(name="mp", bufs=1) as mp,
            tc.tile_pool(name="ptp", bufs=2) as ptp,
            tc.tile_pool(name="stg", bufs=4) as stg,
            tc.tile_pool(name="dacc", bufs=2) as dap,
            tc.tile_pool(name="dr", bufs=1, space="DRAM") as dr,
            tc.tile_pool(name="ps", bufs=1, space="PSUM") as ps,
        ):
            def pbank(o, name):
                return ps.tile([128, 512], F32, tag=f"pb{o}", bufs=1, name=name)

            # ---- PE warm-up: the tensor engine ramps 0.65->2.4GHz over ~3us
            #      of continuous execution; burn the ramp on dummy matmuls
            #      while the first input DMAs are still in flight ----
            wu_src = mp.tile([128, 512], BF16, tag="wusrc", name="wu_src")
            nc.vector.memset(wu_src, 0.0)
            wu_ps = pbank(7, "wu_ps")
            for w in range(6):
                nc.tensor.matmul(
                    wu_ps, lhsT=wu_src[:, 0:128], rhs=wu_src,
                    start=(w == 0), stop=(w == 5),
                )
            nc.vector.tensor_copy(wu_src, wu_ps)

            xt_sb = [
                xp.tile([128, 1024], BF16, tag=f"x{i}", name=f"x{i}")
                for i in range(IC)
            ]
            wq_sb = [
                wp.tile([128, D], BF16, tag=f"wq{i}", name=f"wq{i}") for i in range(IC)
            ]
            wk_sb = [
                wp.tile([128, D], BF16, tag=f"wk{i}", name=f"wk{i}") for i in range(IC)
            ]
            wv_sb = [
                wp.tile([128, D], BF16, tag=f"wv{i}", name=f"wv{i}") for i in range(IC)
            ]
            # wave 1: x on sync queue, Wq on scalar queue, in consumption
            # order; the very first chunks are split so matmul #1 can start
            # as soon as ~300KB has landed
            nc.sync.dma_start(xt_sb[0][:, 0:512], xP[:, 0:512])
            nc.scalar.dma_start(wq_sb[0][:, 0:128], wqP[:, 0:128])
            nc.sync.dma_start(xt_sb[0][:, 512:1024], xP[:, 512:1024])
            nc.scalar.dma_start(wq_sb[0][:, 128:1024], wqP[:, 128:1024])
            for i in range(1, IC):
                nc.sync.dma_start(xt_sb[i], xP[:, 1024 * i : 1024 * (i + 1)])
                nc.scalar.dma_start(wq_sb[i], wqP[:, 1024 * i : 1024 * (i + 1)])
            # wave 2: Wk on scalar, Wv on sync
            for i in range(IC):
                nc.scalar.dma_start(wk_sb[i], wkP[:, 1024 * i : 1024 * (i + 1)])
                nc.sync.dma_start(wv_sb[i], wvP[:, 1024 * i : 1024 * (i + 1)])
            mask_all = mp.tile([128, 2, 128], BF16, tag="masks", name="masks")
            nc.scalar.dma_start(mask_all, mskP.rearrange("p (m c) -> p m c", c=128))

            kt_sb = [kqv.tile([128, S // 2], BF16, tag=f"kt{o}", name=f"kt{o}") for o in range(OC)]
            vn_sb = [kqv.tile([128, D], BF16, tag=f"vn{j}", name=f"vn{j}") for j in range(LC)]
            # own Q staging: doubles as the scores rhs for tiles 0/1
            qs = kqv.tile([128, 2, OC, 512], BF16, tag="qs", name="qs")
            # peer Q, recovered as (own+peer) - own after the AllReduce
            qt_peer = kqv.tile([128, 2, OC, 512], BF16, tag="qtp", name="qt_peer")
            rs_sb = mp.tile([1, S], F32, tag="rs", name="rs")

            # DRAM bounce buffers for the pairwise Q AllReduce
            qhalf = dr.tile([128, 2 * 4096], BF16, name="qhalf")
            qsum = dr.tile([128, 2 * 4096], BF16, name="qsum")

            def copy_to(dst):
                # projection copies go to DVE: the Scalar engine is busy
                # issuing input DMAs early on
                return lambda pp: nc.vector.tensor_copy(dst, pp)

            def proj_iouter(lhs_slices, rhs_slices, dsts, pname):
                pps = [pbank(o, f"{pname}{o}") for o in range(len(dsts))]
                for i in range(IC):
                    for o in range(len(dsts)):
                        nc.tensor.matmul(
                            pps[o],
                            lhsT=lhs_slices(i, o),
                            rhs=rhs_slices(i, o),
                            start=(i == 0),
                            stop=(i == IC - 1),
                        )
                for o, dst in enumerate(dsts):
                    dst(pps[o])

            # ---- phase 1: Q projection (both 512-column halves of own x),
            #      staged to qs; each half's flat copy streams to DRAM for
            #      the pairwise AllReduce as soon as it completes ----
            for st in range(2):
                proj_iouter(
                    lambda i, o: wq_sb[i][:, 128 * o : 128 * (o + 1)],
                    lambda i, o, s=st: xt_sb[i][:, 512 * s : 512 * (s + 1)],
                    [copy_to(qs[:, st, o, :]) for o in range(OC)],
                    f"pq{st}",
                )
                nc.gpsimd.dma_start(
                    qhalf[:, 4096 * st : 4096 * (st + 1)],
                    qs[:, st].rearrange("p o c -> p (o c)"),
                )
            # pairwise exchange: AllReduce(add) then peer = sum - own.
            # Triggered from GpSimd (idle until the attention phase).
            nc.gpsimd.collective_compute(
                kind="AllReduce",
                op=mybir.AluOpType.add,
                replica_groups=[[0, 1], [2, 3], [4, 5], [6, 7]],
                ins=[qhalf],
                outs=[qsum],
            )
            for st in range(2):
                nc.sync.dma_start(
                    qt_peer[:, st],
                    qsum[:, 4096 * st : 4096 * (st + 1)].rearrange(
                        "p (o c) -> p o c", c=512
                    ),
                )
            # NOTE: the peer = sum - own subtracts are emitted just before
            # attention_tile(2): the DVE queue is in-order, and an early
            # subtract would block every projection copy behind the slow
            # AllReduce.

            # K projection (both halves of the local context)
            for st in range(2):
                proj_iouter(
                    lambda i, o: wk_sb[i][:, 128 * o : 128 * (o + 1)],
                    lambda i, o, s=st: xt_sb[i][:, 512 * s : 512 * (s + 1)],
                    [copy_to(kt_sb[o][:, 512 * st : 512 * (st + 1)]) for o in range(OC)],
                    f"pk{st}",
                )
            # V projection for chunks j<4 (all that attention tiles 0 and 2
            # need); chunks j>=4 are projected between tiles 0 and 1
            proj_iouter(
                lambda i, c: xt_sb[i][:, 128 * (c // 2) : 128 * (c // 2 + 1)],
                lambda i, c: wv_sb[i][:, 512 * (c % 2) : 512 * (c % 2 + 1)],
                [
                    (lambda dst: lambda pp: nc.any.tensor_copy(dst, pp))(
                        vn_sb[c // 2][:, 512 * (c % 2) : 512 * (c % 2 + 1)]
                    )
                    for c in range(8)
                ],
                "pva",
            )

            def vn_late():
                for c in range(8):
                    j, ot = 4 + c // 2, c % 2
                    pp = pbank(c, "pvb")
                    for i in range(IC):
                        nc.tensor.matmul(
                            pp,
                            lhsT=xt_sb[i][:, 128 * j : 128 * (j + 1)],
                            rhs=wv_sb[i][:, 512 * ot : 512 * (ot + 1)],
                            start=(i == 0),
                            stop=(i == IC - 1),
                        )
                    if c % 2 == 0:
                        nc.vector.tensor_copy(
                            vn_sb[j][:, 512 * ot : 512 * (ot + 1)], pp
                        )
                    else:
                        nc.scalar.copy(
                            vn_sb[j][:, 512 * ot : 512 * (ot + 1)], pp
                        )

            # ---- phase 2: attention over gathered q-tiles ----
            def attention_tile(tt):
                st = tt % 2
                own = tt < 2
                q_src = qs[:, st] if own else qt_peer[:, st]
                nj = NJ_TILE[tt]
                masked = set(_masked_js(tt))
                mgrp = 0 if own else 1
                pt_tiles = []
                offs = []
                acc = dap.tile([128, 512], F32, tag="dacc", name=f"acc{tt}")
                for j in range(nj):
                    # in a masked block the first 128*(j%4) q-columns are
                    # fully masked out -- skip computing them entirely; the
                    # mask itself only covers the 128-wide diagonal block
                    off = 128 * (j % 4) if j in masked else 0
                    offs.append(off)
                    sp = pbank(j, f"sc{tt}_{j}")
                    for o in range(OC):
                        nc.tensor.matmul(
                            sp[:, off:512],
                            lhsT=kt_sb[o][:, 128 * j : 128 * (j + 1)],
                            rhs=q_src[:, o, off:512],
                            start=(o == 0),
                            stop=(o == OC - 1),
                        )
                    pt = ptp.tile([128, 512], BF16, tag=f"pt{j}", name=f"pt{tt}_{j}")
                    nc.scalar.activation(
                        pt[:, off:512],
                        sp[:, off:512],
                        mybir.ActivationFunctionType.Exp,
                        scale=SCALE,
                    )
                    if j in masked:
                        nc.vector.tensor_mul(
                            pt[:, off : off + 128],
                            pt[:, off : off + 128],
                            mask_all[:, mgrp, :],
                        )
                    pt_tiles.append(pt)
                    # partial softmax denominators accumulate on DVE (fast,
                    # and frees the pt buffer quickly); only the partition
                    # reduction runs on GpSimd (j=0 always covers all 512)
                    if j == 0:
                        nc.vector.tensor_copy(acc, pt)
                    else:
                        nc.vector.tensor_add(
                            acc[:, off:512], acc[:, off:512], pt[:, off:512]
                        )
                red = dap.tile([128, 512], F32, tag="dred", name=f"red{tt}")
                nc.gpsimd.partition_all_reduce(
                    red, acc, channels=128, reduce_op=bass_isa.ReduceOp.add
                )
                nc.vector.tensor_copy(rs_sb[:, 512 * tt : 512 * (tt + 1)], red[0:1, :])

                # AV in ascending causal size: the first blocks depend only on
                # early pt tiles, so they never wait on the last exp
                bank = 0 if nj == 8 else 4
                for qq in range(4):
                    qbg = 4 * tt + qq
                    njs = 4 * st + qq + 1
                    ost = stg.tile([128, D], F32, tag="ost", name=f"ost{tt}_{qq}")
                    for ot in range(2):
                        apsum = pbank(bank % 8, f"at{tt}_{qq}_{ot}")
                        bank += 1
                        for j in range(njs):
                            nc.tensor.matmul(
                                apsum,
                                lhsT=pt_tiles[j][:, 128 * qq : 128 * (qq + 1)],
                                rhs=vn_sb[j][:, 512 * ot : 512 * (ot + 1)],
                                start=(j == 0),
                                stop=(j == njs - 1),
                            )
                        nc.vector.tensor_copy(ost[:, 512 * ot : 512 * (ot + 1)], apsum)
                        if tt == 3:
                            # final tile: split per-half on the idle Scalar
                            # queue so the last DMA is only 256KB
                            nc.scalar.dma_start(
                                out_p[128 * qbg : 128 * (qbg + 1), 512 * ot : 512 * (ot + 1)],
                                ost[:, 512 * ot : 512 * (ot + 1)],
                            )
                    if tt != 3:
                        nc.gpsimd.dma_start(
                            out_p[128 * qbg : 128 * (qbg + 1), :], ost
                        )

            attention_tile(0)
            vn_late()
            attention_tile(1)
            # recover peer Q (sum - own) now that the AllReduce is long done
            for st in range(2):
                nc.vector.tensor_sub(qt_peer[:, st], qt_peer[:, st], qs[:, st])
            attention_tile(2)
            attention_tile(3)

            nc.gpsimd.dma_start(rs_out, rs_sb)

    nc.compile()
    return nc


def _get_module():
    global _module_cache
    if _module_cache is None:
        _module_cache = _build_module()
    return _module_cache


def _gather_pos(par: int) -> np.ndarray:
    """pos[q] = row position of global row q in this core's rank-relative
    gathered order ([own interleaved blocks | peer interleaved blocks])."""
    q = np.arange(S)
    g = q // 128
    r = q % 128
    own = (g % 2) == par
    blk = np.where(own, (g - par) // 2, 8 + (g - (1 - par)) // 2)
    return 128 * blk + r


def _pack_pmajor(a2d: np.ndarray) -> np.ndarray:
    """[IC*128, W] -> [128, IC*W] partition-major packing (bf16)."""
    w = a2d.shape[1]
    return np.ascontiguousarray(
        a2d.reshape(IC, 128, w).transpose(1, 0, 2).reshape(128, IC * w)
    )


def kernel(x, Wq, Wk, Wv, _trace=False):
    global last_results
    nc = _get_module()

    bf = ml_dtypes.bfloat16

    wqP = _pack_pmajor(np.ascontiguousarray(Wq.T).astype(bf))
    wkP = _pack_pmajor(np.ascontiguousarray(Wk.T).astype(bf))
    wvP = _pack_pmajor(np.ascontiguousarray(Wv.T).astype(bf))

    # masks: group 0 = triangular diagonal block (own tiles), group 1 =
    # all-ones (parity 0) / all-zeros (parity 1) for the peer tiles' block
    tri = (np.arange(128)[None, :] >= np.arange(128)[:, None]).astype(np.float32)
    msks = []
    for par in range(2):
        m = np.zeros((128, 2, 128), dtype=np.float32)
        m[:, 0, :] = tri
        m[:, 1, :] = 1.0 - par
        msks.append(m.reshape(128, 256).astype(bf))

    # per-parity column selection: core owns global k-chunks {2j+par}
    own_cols = [
        (128 * (2 * np.arange(LC)[:, None] + par) + np.arange(128)[None, :]).reshape(-1)
        for par in range(2)
    ]

    in_maps = []
    for c in range(8):
        b, par = c // 2, c % 2
        xTb = x[b].T[:, own_cols[par]].astype(bf)  # [D, S//2]
        in_maps.append(
            {
                "xP": _pack_pmajor(xTb),
                "wqP": wqP,
                "wkP": wkP,
                "wvP": wvP,
                "mskP": msks[par],
            }
        )

    kwargs = {}
    if _trace:
        kwargs["trace"] = True
    res = run_bass_kernel_spmd(nc, in_maps, core_ids=list(range(8)), **kwargs)
    last_results = res

    pos = [_gather_pos(0), _gather_pos(1)]
    out = np.empty((B, S, D), dtype=np.float32)
    for b in range(B):
        rA = res.results[2 * b]
        rB = res.results[2 * b + 1]
        num = rA["out_p"][pos[0]] + rB["out_p"][pos[1]]
        den = rA["rs_out"][0][pos[0]] + rB["rs_out"][0][pos[1]]
        out[b] = num / den[:, None]
    return out


# revision 14
# speedup vs baseline: 1.3459x; 1.1129x over previous
"""Causal attention (B=4, S=2048, D=1024, single head) on 8 TRN2 NeuronCores.

Sharding: data-parallel over batch (4 pairs of cores); within each pair
the K/V context is split by interleaved 128-row chunks (core parity p
owns global chunks {2j+p}).  Each core projects Q/K/V for its own 1024
rows.  Attention q-columns are processed in a RANK-RELATIVE gathered
order [own 8 blocks | peer 8 blocks]: the own half's Q is read straight
from the SBUF staging tile (no communication), and the peer half is
recovered from a pairwise AllReduce(add) of the Q staging buffers via
one local subtract (peer = sum - own, exact to ~1 ulp bf16).  Each core
computes causal score blocks of all 2048 gathered q-columns against its
own 1024-row context and emits *unnormalized* partial attention output
plus per-column partial softmax denominators (computed off the PE on
GpSimd via partition_all_reduce); the host adds the pair's partials and
normalizes.

PE-stream optimizations: dummy warm-up matmuls cover the tensor-engine
p-state ramp; a single PSUM pool with per-bank tags avoids pool-
transition drains; inputs are host-packed partition-major so every DMA
moves 2KB+ per-partition lines; causal masks cover only the 128-column
diagonal block of each masked score tile; AV blocks run in ascending
causal size so they never wait on the last exp; the final tile's output
DMAs are split per 512-column half to shrink the tail.
"""

import sys

if "/opt/trn_rl_repo" not in sys.path:
    sys.path.insert(0, "/opt/trn_rl_repo")

import ml_dtypes
import numpy as np

import concourse.bacc as bacc
import concourse.bass_isa as bass_isa
import concourse.tile as tile
from concourse import mybir
from concourse.bass_utils import run_bass_kernel_spmd

# bass_utils imports antenv.axon_hooks when tracing is requested (e.g. via a
# BASS_TRACE env var); the image's antenv lacks that module, so provide a
# no-op fallback rather than crashing.
try:
    import antenv.axon_hooks  # noqa: F401
except ImportError:
    import types as _types

    _ah = _types.ModuleType("antenv.axon_hooks")
    _ah._hook = None
    _ah.set_axon_ntff_profile_hook = lambda h: setattr(_ah, "_hook", h)
    _ah.get_axon_ntff_profile_hook = lambda: _ah._hook
    sys.modules["antenv.axon_hooks"] = _ah

B, S, D = 4, 2048, 1024
NB = S // 128          # 16 q-blocks of 128 per batch
NT = S // 512          # 4 q-tiles of 512
IC = D // 128          # 8 contraction chunks
OC = D // 128          # 8 output-dim chunks
LC = 8                 # local k-chunks per core (S/2/128)
SCALE = 1.0 / np.sqrt(D)  # 0.03125
# gathered tiles: 0 = own st0, 1 = own st1, 2 = peer st0, 3 = peer st1
NJ_TILE = [4, 8, 4, 8]  # local k-chunks needed per gathered q-tile

BF16 = mybir.dt.bfloat16
F32 = mybir.dt.float32

_module_cache = None
last_results = None  # BassKernelResults of the most recent run (for test harness)


def _masked_js(tt):
    """Local chunk indices whose score blocks carry an offset+mask."""
    return range(4) if tt in (0, 2) else range(4, 8)


def _build_module():
    nc = bacc.Bacc("TRN2", target_bir_lowering=False, debug=False, num_devices=8)
    # all inputs packed partition-major on the host: per-partition lines are
    # 2KB+ contiguous so every input DMA runs at full HBM efficiency
    xP = nc.dram_tensor("xP", [128, IC * 1024], BF16, kind="ExternalInput").ap()
    wqP = nc.dram_tensor("wqP", [128, IC * 1024], BF16, kind="ExternalInput").ap()
    wkP = nc.dram_tensor("wkP", [128, IC * 1024], BF16, kind="ExternalInput").ap()
    wvP = nc.dram_tensor("wvP", [128, IC * 1024], BF16, kind="ExternalInput").ap()
    mskP = nc.dram_tensor("mskP", [128, 2 * 128], BF16, kind="ExternalInput").ap()
    out_p = nc.dram_tensor("out_p", [S, D], F32, kind="ExternalOutput").ap()
    rs_out = nc.dram_tensor("rs_out", [1, S], F32, kind="ExternalOutput").ap()

    with tile.TileContext(nc) as tc:
        with (
            tc.tile_pool(name="wp", bufs=1) as wp,
            tc.tile_pool(name="xp", bufs=1) as xp,
            tc.tile_pool(name="kqv", bufs=1) as kqv,
            tc.tile_pool(name="mp", bufs=1) as mp,
            tc.tile_pool(name="ptp", bufs=2) as ptp,
            tc.tile_pool(name="stg", bufs=4) as stg,
            tc.tile_pool(name="dacc", bufs=2) as dap,
            tc.tile_pool(name="dr", bufs=1, space="DRAM") as dr,
            tc.tile_pool(name="ps", bufs=1, space="PSUM") as ps,
        ):
            def pbank(o, name):
                return ps.tile([128, 512], F32, tag=f"pb{o}", bufs=1, name=name)

            # ---- PE warm-up: the tensor engine ramps 0.65->2.4GHz over ~3us
            #      of continuous execution; burn the ramp on dummy matmuls
            #      while the first input DMAs are still in flight ----
            wu_src = mp.tile([128, 512], BF16, tag="wusrc", name="wu_src")
            nc.vector.memset(wu_src, 0.0)
            wu_ps = pbank(7, "wu_ps")
            for w in range(6):
                nc.tensor.matmul(
                    wu_ps, lhsT=wu_src[:, 0:128], rhs=wu_src,
                    start=(w == 0), stop=(w == 5),
                )
            nc.vector.tensor_copy(wu_src, wu_ps)

            xt_sb = [
                xp.tile([128, 1024], BF16, tag=f"x{i}", name=f"x{i}")
                for i in range(IC)
            ]
            wq_sb = [
                wp.tile([128, D], BF16, tag=f"wq{i}", name=f"wq{i}") for i in range(IC)
            ]
            wk_sb = [
                wp.tile([128, D], BF16, tag=f"wk{i}", name=f"wk{i}") for i in range(IC)
            ]
            wv_sb = [
                wp.tile([128, D], BF16, tag=f"wv{i}", name=f"wv{i}") for i in range(IC)
            ]
            # wave 1: x on sync queue, Wq on scalar queue, in consumption
            # order; the very first chunks are split so matmul #1 can start
            # as soon as ~300KB has landed
            nc.sync.dma_start(xt_sb[0][:, 0:512], xP[:, 0:512])
            nc.scalar.dma_start(wq_sb[0][:, 0:128], wqP[:, 0:128])
            nc.sync.dma_start(xt_sb[0][:, 512:1024], xP[:, 512:1024])
            nc.scalar.dma_start(wq_sb[0][:, 128:1024], wqP[:, 128:1024])
            for i in range(1, IC):
                nc.sync.dma_start(xt_sb[i], xP[:, 1024 * i : 1024 * (i + 1)])
                nc.scalar.dma_start(wq_sb[i], wqP[:, 1024 * i : 1024 * (i + 1)])
            # wave 2: Wk on scalar, Wv on sync
            for i in range(IC):
                nc.scalar.dma_start(wk_sb[i], wkP[:, 1024 * i : 1024 * (i + 1)])
                nc.sync.dma_start(wv_sb[i], wvP[:, 1024 * i : 1024 * (i + 1)])
            mask_all = mp.tile([128, 2, 128], BF16, tag="masks", name="masks")
            nc.scalar.dma_start(mask_all, mskP.rearrange("p (m c) -> p m c", c=128))

            kt_sb = [kqv.tile([128, S // 2], BF16, tag=f"kt{o}", name=f"kt{o}") for o in range(OC)]
            vn_sb = [kqv.tile([128, D], BF16, tag=f"vn{j}", name=f"vn{j}") for j in range(LC)]
            # own Q staging: doubles as the scores rhs for tiles 0/1
            qs = kqv.tile([128, 2, OC, 512], BF16, tag="qs", name="qs")
            # negated own Q: peer = (own+peer) + (-own); computed on DVE right
            # after the projection (dependency-free), added on GpSimd (whose
            # ucode implements Add but not Subtract)
            nqs = kqv.tile([128, 2, OC, 512], BF16, tag="nqs", name="nqs")
            # peer Q, recovered from the AllReduce sum
            qt_peer = kqv.tile([128, 2, OC, 512], BF16, tag="qtp", name="qt_peer")
            rs_sb = mp.tile([1, S], F32, tag="rs", name="rs")

            # DRAM bounce buffers for the pairwise Q AllReduce (one per
            # 512-column half so the first exchange starts ASAP)
            qhalf = [dr.tile([128, 4096], BF16, name=f"qhalf{st}") for st in range(2)]
            qsum = [dr.tile([128, 4096], BF16, name=f"qsum{st}") for st in range(2)]

            def copy_to(dst):
                # projection copies go to DVE: the Scalar engine is busy
                # issuing input DMAs early on
                return lambda pp: nc.vector.tensor_copy(dst, pp)

            def proj_iouter(lhs_slices, rhs_slices, dsts, pname):
                pps = [pbank(o, f"{pname}{o}") for o in range(len(dsts))]
                for i in range(IC):
                    for o in range(len(dsts)):
                        nc.tensor.matmul(
                            pps[o],
                            lhsT=lhs_slices(i, o),
                            rhs=rhs_slices(i, o),
                            start=(i == 0),
                            stop=(i == IC - 1),
                        )
                for o, dst in enumerate(dsts):
                    dst(pps[o])

            # ---- phase 1: Q projection (both 512-column halves of own x),
            #      staged to qs; each half's flat copy streams to DRAM for
            #      the pairwise AllReduce as soon as it completes ----
            # pairwise exchange per half: AllReduce(add) then peer = sum -
            # own.  Everything in the exchange chain runs on queues with no
            # other pending work (gpsimd/sync), so the slow collective never
            # blocks the projection/attention pipeline.
            for st in range(2):
                proj_iouter(
                    lambda i, o: wq_sb[i][:, 128 * o : 128 * (o + 1)],
                    lambda i, o, s=st: xt_sb[i][:, 512 * s : 512 * (s + 1)],
                    [copy_to(qs[:, st, o, :]) for o in range(OC)],
                    f"pq{st}",
                )
                nc.vector.tensor_scalar_mul(nqs[:, st], qs[:, st], -1.0)
                nc.gpsimd.dma_start(
                    qhalf[st], qs[:, st].rearrange("p o c -> p (o c)")
                )
                nc.gpsimd.collective_compute(
                    kind="AllReduce",
                    op=mybir.AluOpType.add,
                    replica_groups=[[0, 1], [2, 3], [4, 5], [6, 7]],
                    ins=[qhalf[st]],
                    outs=[qsum[st]],
                )
            for st in range(2):
                nc.sync.dma_start(
                    qt_peer[:, st],
                    qsum[st].rearrange("p (o c) -> p o c", c=512),
                )
                nc.gpsimd.tensor_add(qt_peer[:, st], qt_peer[:, st], nqs[:, st])

            # K projection (both halves of the local context)
            for st in range(2):
                proj_iouter(
                    lambda i, o: wk_sb[i][:, 128 * o : 128 * (o + 1)],
                    lambda i, o, s=st: xt_sb[i][:, 512 * s : 512 * (s + 1)],
                    [copy_to(kt_sb[o][:, 512 * st : 512 * (st + 1)]) for o in range(OC)],
                    f"pk{st}",
                )
            # V projection for chunks j<4 (all that attention tiles 0 and 2
            # need); chunks j>=4 are projected between tiles 0 and 1
            proj_iouter(
                lambda i, c: xt_sb[i][:, 128 * (c // 2) : 128 * (c // 2 + 1)],
                lambda i, c: wv_sb[i][:, 512 * (c % 2) : 512 * (c % 2 + 1)],
                [
                    (lambda dst: lambda pp: nc.any.tensor_copy(dst, pp))(
                        vn_sb[c // 2][:, 512 * (c % 2) : 512 * (c % 2 + 1)]
                    )
                    for c in range(8)
                ],
                "pva",
            )

            def vn_late():
                for c in range(8):
                    j, ot = 4 + c // 2, c % 2
                    pp = pbank(c, "pvb")
                    for i in range(IC):
                        nc.tensor.matmul(
                            pp,
                            lhsT=xt_sb[i][:, 128 * j : 128 * (j + 1)],
                            rhs=wv_sb[i][:, 512 * ot : 512 * (ot + 1)],
                            start=(i == 0),
                            stop=(i == IC - 1),
                        )
                    if c % 2 == 0:
                        nc.vector.tensor_copy(
                            vn_sb[j][:, 512 * ot : 512 * (ot + 1)], pp
                        )
                    else:
                        nc.scalar.copy(
                            vn_sb[j][:, 512 * ot : 512 * (ot + 1)], pp
                        )

            # ---- phase 2: attention over gathered q-tiles ----
            def attention_tile(tt):
                st = tt % 2
                own = tt < 2
                q_src = qs[:, st] if own else qt_peer[:, st]
                nj = NJ_TILE[tt]
                masked = set(_masked_js(tt))
                mgrp = 0 if own else 1
                pt_tiles = []
                offs = []
                acc = dap.tile([128, 512], F32, tag="dacc", name=f"acc{tt}")
                for j in range(nj):
                    # in a masked block the first 128*(j%4) q-columns are
                    # fully masked out -- skip computing them entirely; the
                    # mask itself only covers the 128-wide diagonal block
                    off = 128 * (j % 4) if j in masked else 0
                    offs.append(off)
                    sp = pbank(j, f"sc{tt}_{j}")
                    for o in range(OC):
                        nc.tensor.matmul(
                            sp[:, off:512],
                            lhsT=kt_sb[o][:, 128 * j : 128 * (j + 1)],
                            rhs=q_src[:, o, off:512],
                            start=(o == 0),
                            stop=(o == OC - 1),
                        )
                    pt = ptp.tile([128, 512], BF16, tag=f"pt{j}", name=f"pt{tt}_{j}")
                    nc.scalar.activation(
                        pt[:, off:512],
                        sp[:, off:512],
                        mybir.ActivationFunctionType.Exp,
                        scale=SCALE,
                    )
                    if j in masked:
                        nc.vector.tensor_mul(
                            pt[:, off : off + 128],
                            pt[:, off : off + 128],
                            mask_all[:, mgrp, :],
                        )
                    pt_tiles.append(pt)
                    # partial softmax denominators accumulate on DVE (fast,
                    # and frees the pt buffer quickly); only the partition
                    # reduction runs on GpSimd (j=0 always covers all 512)
                    if j == 0:
                        nc.vector.tensor_copy(acc, pt)
                    else:
                        nc.vector.tensor_add(
                            acc[:, off:512], acc[:, off:512], pt[:, off:512]
                        )
                red = dap.tile([128, 512], F32, tag="dred", name=f"red{tt}")
                nc.gpsimd.partition_all_reduce(
                    red, acc, channels=128, reduce_op=bass_isa.ReduceOp.add
                )
                nc.vector.tensor_copy(rs_sb[:, 512 * tt : 512 * (tt + 1)], red[0:1, :])

                # AV in ascending causal size: the first blocks depend only on
                # early pt tiles, so they never wait on the last exp
                bank = 0 if nj == 8 else 4
                for qq in range(4):
                    qbg = 4 * tt + qq
                    njs = 4 * st + qq + 1
                    ost = stg.tile([128, D], F32, tag="ost", name=f"ost{tt}_{qq}")
                    for ot in range(2):
                        apsum = pbank(bank % 8, f"at{tt}_{qq}_{ot}")
                        bank += 1
                        for j in range(njs):
                            nc.tensor.matmul(
                                apsum,
                                lhsT=pt_tiles[j][:, 128 * qq : 128 * (qq + 1)],
                                rhs=vn_sb[j][:, 512 * ot : 512 * (ot + 1)],
                                start=(j == 0),
                                stop=(j == njs - 1),
                            )
                        nc.vector.tensor_copy(ost[:, 512 * ot : 512 * (ot + 1)], apsum)
                        if tt == 3:
                            # final tile: split per-half on the idle Scalar
                            # queue so the last DMA is only 256KB
                            nc.scalar.dma_start(
                                out_p[128 * qbg : 128 * (qbg + 1), 512 * ot : 512 * (ot + 1)],
                                ost[:, 512 * ot : 512 * (ot + 1)],
                            )
                    if tt != 3:
                        nc.gpsimd.dma_start(
                            out_p[128 * qbg : 128 * (qbg + 1), :], ost
                        )

            attention_tile(0)
            vn_late()
            attention_tile(1)
            attention_tile(2)
            attention_tile(3)

            nc.gpsimd.dma_start(rs_out, rs_sb)

    nc.compile()
    return nc


def _get_module():
    global _module_cache
    if _module_cache is None:
        _module_cache = _build_module()
    return _module_cache


def _gather_pos(par: int) -> np.ndarray:
    """pos[q] = row position of global row q in this core's rank-relative
    gathered order ([own interleaved blocks | peer interleaved blocks])."""
    q = np.arange(S)
    g = q // 128
    r = q % 128
    own = (g % 2) == par
    blk = np.where(own, (g - par) // 2, 8 + (g - (1 - par)) // 2)
    return 128 * blk + r


def _pack_pmajor(a2d: np.ndarray) -> np.ndarray:
    """[IC*128, W] -> [128, IC*W] partition-major packing (bf16)."""
    w = a2d.shape[1]
    return np.ascontiguousarray(
        a2d.reshape(IC, 128, w).transpose(1, 0, 2).reshape(128, IC * w)
    )


def kernel(x, Wq, Wk, Wv, _trace=False):
    global last_results
    nc = _get_module()

    bf = ml_dtypes.bfloat16

    wqP = _pack_pmajor(np.ascontiguousarray(Wq.T).astype(bf))
    wkP = _pack_pmajor(np.ascontiguousarray(Wk.T).astype(bf))
    wvP = _pack_pmajor(np.ascontiguousarray(Wv.T).astype(bf))

    # masks: group 0 = triangular diagonal block (own tiles), group 1 =
    # all-ones (parity 0) / all-zeros (parity 1) for the peer tiles' block
    tri = (np.arange(128)[None, :] >= np.arange(128)[:, None]).astype(np.float32)
    msks = []
    for par in range(2):
        m = np.zeros((128, 2, 128), dtype=np.float32)
        m[:, 0, :] = tri
        m[:, 1, :] = 1.0 - par
        msks.append(m.reshape(128, 256).astype(bf))

    # per-parity column selection: core owns global k-chunks {2j+par}
    own_cols = [
        (128 * (2 * np.arange(LC)[:, None] + par) + np.arange(128)[None, :]).reshape(-1)
        for par in range(2)
    ]

    in_maps = []
    for c in range(8):
        b, par = c // 2, c % 2
        xTb = x[b].T[:, own_cols[par]].astype(bf)  # [D, S//2]
        in_maps.append(
            {
                "xP": _pack_pmajor(xTb),
                "wqP": wqP,
                "wkP": wkP,
                "wvP": wvP,
                "mskP": msks[par],
            }
        )

    kwargs = {}
    if _trace:
        kwargs["trace"] = True
    res = run_bass_kernel_spmd(nc, in_maps, core_ids=list(range(8)), **kwargs)
    last_results = res

    pos = [_gather_pos(0), _gather_pos(1)]
    out = np.empty((B, S, D), dtype=np.float32)
    for b in range(B):
        rA = res.results[2 * b]
        rB = res.results[2 * b + 1]
        num = rA["out_p"][pos[0]] + rB["out_p"][pos[1]]
        den = rA["rs_out"][0][pos[0]] + rB["rs_out"][0][pos[1]]
        out[b] = num / den[:, None]
    return out
